# revision 1
# baseline (speedup 1.0000x reference)
"""Conv3x3(8->64) + GroupNorm(16) + scale + MaxPool4 + clamp kernel for TRN2.

Layout summary (per core, S samples):
  im2col I4[(ic,r,kw)=96 part, pair=63, w=126] fp32, pair covers out rows (2p, 2p+1)
  conv: 16 matmuls per sample, lhsT=W96[96,128] (f32r), rhs=I4 slice, N=504/378
        PSUM tile [128=(oc+64j), 1008] holds 2 matmuls
  ACT per psum tile: Identity + per-partition bias, accum_out=partial sum(y),
        out -> Ybf2 fp16 [128, pair*128 + par*64 + w2] (w parity interleave, w2 padded to 64)
  DVE: sumsq via scalar_tensor_tensor accum; maxpool as TT-max tree
        lvl1 (par), lvl2 (w2 pairs), hpool (row pairs), jmax (partition halves)
  stats: SEL matmul -> per-channel group sums; tiny ops -> A, B(neg)
  finalize: ACT Relu(A*p + B) then DVE min(.,1) cast fp32; DMA out
"""

import numpy as np
import concourse.bass as bass
import concourse.tile as tile
from concourse import bacc, mybir
from contextlib import ExitStack

F32 = mybir.dt.float32
F32R = mybir.dt.float32r
F16 = mybir.dt.float16

EPS = 1e-5
NPIX = 4 * 126 * 126  # elements per (sample, group)
NEG_INF = float("-inf")


def _ap(base, dims, offset):
    """Copy of AP `base` with raw [step,count] dims and element offset."""
    a = base.copy()
    a.ap = mybir.VecI64Pair([list(d) for d in dims])
    a.offset = offset
    return a


def _apf(base, free_dims, elem_offset):
    """SBUF AP: keep `base`'s partition dim, replace free dims, add offset."""
    a = base.copy()
    a.ap = mybir.VecI64Pair([list(base.ap[0])] + [list(d) for d in free_dims])
    a.offset = base.offset + elem_offset
    return a


class Pools:
    pass


def build_pools(ctx, tc):
    p = Pools()
    p.consts = ctx.enter_context(tc.tile_pool(name="consts", bufs=1))
    p.i4 = ctx.enter_context(tc.tile_pool(name="i4", bufs=2))
    p.ps = ctx.enter_context(tc.tile_pool(name="psc", bufs=3, space="PSUM"))
    p.gps = ctx.enter_context(tc.tile_pool(name="gps", bufs=1, space="PSUM"))
    p.y = ctx.enter_context(tc.tile_pool(name="ybuf", bufs=2))
    p.pool = ctx.enter_context(tc.tile_pool(name="pools", bufs=2))
    p.st = ctx.enter_context(tc.tile_pool(name="stats", bufs=2))
    return p


def load_consts(nc, p, w96_in, sel_in, bias_in, ws_in, gb_in):
    c = Pools()
    wkt = p.consts.tile([32, 384], F32R, tag="w96")
    nc.sync.dma_start(wkt[:], w96_in[:])
    c.wk = [wkt[:, 128 * kw: 128 * kw + 128] for kw in range(3)]
    c.sel = p.consts.tile([128, 64], F32, tag="sel")
    nc.sync.dma_start(c.sel[:], sel_in[:])
    c.cbias = p.consts.tile([128, 1], F32, tag="cbias")
    nc.sync.dma_start(c.cbias[:], bias_in[:])
    c.ws = p.consts.tile([64, 1], F32, tag="ws")
    nc.sync.dma_start(c.ws[:], ws_in[:])
    c.gb = p.consts.tile([64, 1], F32, tag="gb")
    nc.sync.dma_start(c.gb[:], gb_in[:])
    return c


def sample_body(nc, tc, p, c, x_in, y_out, n):
    AL = mybir.AluOpType
    AF = mybir.ActivationFunctionType

    # --- X4s: 4 row-shifted full copies of x[n]: X4s[(ic,r), row*128+w]
    # = x[n, ic, row+r, w]. One DMA, 32 contiguous ~62KB descriptors.
    i4 = p.i4.tile([32, 16000], F32R, tag="i4")
    base = i4[:]
    pstep = base.ap[0][0]
    dst = _ap(base, [[pstep, 32], [1, 16000]], base.offset)
    src = _ap(x_in, [[16384, 8], [128, 4], [1, 16000]], n * 131072)
    nc.sync.dma_start(dst, src)

    # --- conv matmuls + ACT copy/bias/accum
    ybf = p.y.tile([128, 8064], F16, tag="ybf")
    # poison the w2=63 pad columns once per sample (used by lvl1 max)
    nc.vector.memset(_apf(ybf[:], [[128, 63], [64, 2]], 63), NEG_INF)
    sacc = p.st.tile([128, 9], F32, tag="sacc")

    # kw-major over groups of 2 psum tiles (4 mm-tiles) to amortize weight loads
    pss = []
    for k in range(8):
        ps = p.ps.tile([128, 1024], F32, tag="ps")  # 2 banks, 512-aligned slots
        pss.append(ps)
        if k % 2 == 1 or k == 7:
            grp = pss[-2:] if k % 2 == 1 else pss[-1:]
            k0 = k - len(grp) + 1
            for kw in range(3):
                for kk, psg in enumerate(grp):
                    for h in range(2):
                        t = 2 * (k0 + kk) + h
                        n_t = 378 if t == 15 else 504
                        npair = 3 if t == 15 else 4
                        rhs = _apf(i4[:], [[256, npair], [1, 126]],
                                   t * 1024 + kw)
                        nc.tensor.matmul(psg[:, 512 * h: 512 * h + n_t],
                                         c.wk[kw], rhs,
                                         start=(kw == 0), stop=(kw == 2))
    for k in range(8):
        ps = pss[k]
        if k < 7:
            # one ACT over both slots: iterate (slot, pair, w)
            in_ap = _apf(ps[:], [[512, 2], [126, 4], [1, 126]], 0)
            out_ap = _apf(ybf[:], [[128, 8], [1, 63], [64, 2]], 1024 * k)
            nc.scalar.activation(out_ap, in_ap, AF.Identity,
                                 bias=c.cbias[:, 0:1], scale=1.0,
                                 accum_out=sacc[:, k: k + 1])
        else:
            # t=14: 4 pairs, t=15: 3 pairs -> two ACT ops
            in_ap = _apf(ps[:], [[126, 4], [1, 126]], 0)
            out_ap = _apf(ybf[:], [[128, 4], [1, 63], [64, 2]], 1024 * 7)
            nc.scalar.activation(out_ap, in_ap, AF.Identity,
                                 bias=c.cbias[:, 0:1], scale=1.0,
                                 accum_out=sacc[:, 7:8])
            in_ap = _apf(ps[:], [[126, 3], [1, 126]], 512)
            out_ap = _apf(ybf[:], [[128, 3], [1, 63], [64, 2]], 1024 * 7 + 4 * 128)
            nc.scalar.activation(out_ap, in_ap, AF.Identity,
                                 bias=c.cbias[:, 0:1], scale=1.0,
                                 accum_out=sacc[:, 8:9])

    stat2 = p.st.tile([128, 2], F32, tag="stat2")

    # --- maxpool tree (fp16)
    p1 = p.pool.tile([128, 4032], F16, tag="p1")
    in0 = _apf(ybf[:], [[128, 63], [1, 64]], 0)
    in1 = _apf(ybf[:], [[128, 63], [1, 64]], 64)
    o1 = _apf(p1[:], [[64, 63], [1, 32], [32, 2]], 0)
    nc.vector.tensor_tensor(o1, in0, in1, op=AL.max)

    in0 = _apf(p1[:], [[64, 63], [1, 32]], 0)
    in1 = _apf(p1[:], [[64, 63], [1, 32]], 32)
    nc.vector.tensor_tensor(in0, in0, in1, op=AL.max)

    p3 = p.pool.tile([128, 961], F16, tag="p3")
    in0 = _apf(p1[:], [[128, 31], [1, 31]], 0)
    in1 = _apf(p1[:], [[128, 31], [1, 31]], 64)
    nc.vector.tensor_tensor(p3[:], in0, in1, op=AL.max)

    # partitions are lane-locked for DVE: remap upper half down via DMA first
    p3b = p.pool.tile([64, 961], F16, tag="p3b")
    nc.sync.dma_start(p3b[:], p3[64:128, :])
    p4 = p.pool.tile([64, 961], F16, tag="p4")
    nc.vector.tensor_tensor(p4[:], p3[0:64, :], p3b[:], op=AL.max)

    # --- sumsq in-place y*y into ybf; ordered after the pool reads (WAR)
    yv = _apf(ybf[:], [[128, 63], [64, 2], [1, 63]], 0)
    nc.vector.scalar_tensor_tensor(yv, yv, 1.0, yv, op0=AL.mult,
                                   op1=AL.mult, accum_out=stat2[:, 1:2])

    # --- stats -> A, B
    nc.vector.reduce_sum(stat2[:, 0:1], sacc[:], axis=mybir.AxisListType.X)
    gsum = p.gps.tile([64, 2], F32, tag="gsum")
    nc.tensor.matmul(gsum[:], c.sel[:], stat2[:], start=True, stop=True)

    mv = p.st.tile([64, 2], F32, tag="mv")
    nc.vector.tensor_scalar(mv[:], gsum[:], 1.0 / NPIX, None, op0=AL.mult)
    msq = p.st.tile([64, 1], F32, tag="msq")
    nc.vector.tensor_tensor(msq[:], mv[:, 0:1], mv[:, 0:1], op=AL.mult)
    veps = p.st.tile([64, 1], F32, tag="veps")
    nc.vector.scalar_tensor_tensor(veps[:], mv[:, 1:2], EPS, msq[:],
                                   op0=AL.add, op1=AL.subtract)
    rv = p.st.tile([64, 1], F32, tag="rv")
    nc.vector.reciprocal(rv[:], veps[:])
    istd = p.st.tile([64, 1], F32, tag="istd")
    nc.scalar.activation(istd[:], rv[:], AF.Sqrt)
    aap = p.st.tile([64, 1], F32, tag="aap")
    nc.vector.tensor_tensor(aap[:], c.ws[:], istd[:], op=AL.mult)
    mua = p.st.tile([64, 1], F32, tag="mua")
    nc.vector.tensor_tensor(mua[:], mv[:, 0:1], aap[:], op=AL.mult)
    bap = p.st.tile([64, 1], F32, tag="bap")
    nc.vector.tensor_tensor(bap[:], c.gb[:], mua[:], op=AL.subtract)

    # --- finalize: relu(A*p + B) then min(.,1) -> fp32
    fin = p4
    nc.vector.tensor_scalar(p4[:], p4[:], aap[:, 0:1], bap[:, 0:1],
                            op0=AL.mult, op1=AL.add)
    outb = p.pool.tile([64, 961], F16, tag="outb")
    nc.vector.tensor_scalar(outb[:], fin[:], 0.0, 1.0, op0=AL.max, op1=AL.min)

    dst = _ap(y_out, [[961, 64], [1, 961]], n * 61504)
    nc.gpsimd.dma_start(dst, outb[:])



def build_kernel_nc(S, n_cores=8, repeat=1, use_for_i=False):
    nc = bacc.Bacc("TRN2", target_bir_lowering=False, debug=False,
                   num_devices=n_cores)
    x_in = nc.dram_tensor("x", [S, 8, 128, 128], F32R, kind="ExternalInput").ap()
    w96_in = nc.dram_tensor("w96", [32, 384], F32R, kind="ExternalInput").ap()
    sel_in = nc.dram_tensor("sel", [128, 64], F32, kind="ExternalInput").ap()
    bias_in = nc.dram_tensor("cbias", [128, 1], F32, kind="ExternalInput").ap()
    ws_in = nc.dram_tensor("ws", [64, 1], F32, kind="ExternalInput").ap()
    gb_in = nc.dram_tensor("gb", [64, 1], F32, kind="ExternalInput").ap()
    y_out = nc.dram_tensor("y", [S, 64, 31, 31], F32, kind="ExternalOutput").ap()
    with tile.TileContext(nc) as tc:
        with ExitStack() as ctx:
            p = build_pools(ctx, tc)
            c = load_consts(nc, p, w96_in, sel_in, bias_in, ws_in, gb_in)
            if use_for_i and repeat > 1:
                with tc.For_i(0, repeat, 1):
                    for n in range(S):
                        sample_body(nc, tc, p, c, x_in, y_out, n)
            else:
                for _ in range(repeat):
                    for n in range(S):
                        sample_body(nc, tc, p, c, x_in, y_out, n)
    nc.compile()
    return nc


def make_consts(conv_w, conv_b, gn_w, gn_b, scale):
    """Host-side constant assembly (all fp32 numpy)."""
    # wk[kw][(ic*4+r), oc+64j] = conv_w[oc, ic, r-j, kw], packed [32, 3*128]
    w96 = np.zeros((32, 384), np.float32)
    oc = np.arange(64)
    for kw in range(3):
        for j in range(2):
            for ic in range(8):
                for kh in range(3):
                    w96[ic * 4 + kh + j, kw * 128 + oc + 64 * j] = \
                        conv_w[oc, ic, kh, kw]
    sel = np.zeros((128, 64), np.float32)
    for j in range(2):
        for o in range(64):
            sel[o + 64 * j, (o // 4) * 4: (o // 4) * 4 + 4] = 1.0
    cbias = np.tile(conv_b.reshape(64, 1), (2, 1)).astype(np.float32)
    ws = (gn_w.reshape(64) * scale.reshape(64)).reshape(64, 1).astype(np.float32)
    gb = (gn_b.reshape(64) * scale.reshape(64)).reshape(64, 1).astype(np.float32)
    return dict(w96=w96, sel=sel, cbias=cbias, ws=ws, gb=gb)


# ---------------------------------------------------------------------------
# Harness entry point: full (unsharded) inputs -> full output.
# ---------------------------------------------------------------------------
N_CORES = 8
S_PER_CORE = 16
_NC_CACHE = {}


def _get_nc(repeat=1, use_for_i=False):
    key = (repeat, use_for_i)
    if key not in _NC_CACHE:
        _NC_CACHE[key] = build_kernel_nc(S_PER_CORE, n_cores=N_CORES,
                                         repeat=repeat, use_for_i=use_for_i)
    return _NC_CACHE[key]


def kernel(x, conv_w, conv_b, gn_w, gn_b, scale):
    from concourse.bass_utils import run_bass_kernel_spmd
    x = np.ascontiguousarray(np.asarray(x), dtype=np.float32)
    consts = make_consts(np.asarray(conv_w, dtype=np.float32),
                         np.asarray(conv_b, dtype=np.float32),
                         np.asarray(gn_w, dtype=np.float32),
                         np.asarray(gn_b, dtype=np.float32),
                         np.asarray(scale, dtype=np.float32))
    nc = _get_nc()
    in_maps = []
    for c in range(N_CORES):
        m = dict(consts)
        m["x"] = x[c * S_PER_CORE:(c + 1) * S_PER_CORE]
        in_maps.append(m)
    res = run_bass_kernel_spmd(nc, in_maps, core_ids=list(range(N_CORES)))
    return np.concatenate([res.results[c]["y"] for c in range(N_CORES)],
                          axis=0)



# revision 6
# speedup vs baseline: 3.1154x; 3.1154x over previous
"""Conv3x3(8->64) + GroupNorm(16) + scale + MaxPool4 + clamp kernel for TRN2.

v2 layout (per core, S samples), redesigned from trace analysis:
  - x cast to bf16 on host; DMA loads quartered im2col i4[128, 3968]:
    partition (g*32 + ic*4 + r) holds rows 32g+r.., so 4 row-groups of the
    PE array run concurrent K=32 matmuls (tile_position auto-derived).
  - conv: bf16 matmuls, N=504, 3-kw PSUM accumulation, 8 tiles/sample.
  - ACT: 2 parity ops per psum tile (src PSUM step-2, dst CONTIGUOUS runs)
    -> ybf fp16 [128, 8064] = [pair 63][parity 2][w2 64], pads zeroed.
    accum_out -> per-channel partial sums.
  - pool (all unit-stride where possible): lvl1 w-pairs (2x), hpool row
    pairs (2x), lvl2 w-quads (1x, small), partition fold via DMA + max.
  - sumsq: single contiguous STT over [128, 8064] (2x mode), zeros in pads.
  - stats: SEL matmul -> group sums; small-op chain; ACT Rsqrt; finalize
    A*p+B, clamp on DVE; gpsimd DMA casts fp16->fp32 out.
"""

import numpy as np
import concourse.bass as bass
import concourse.tile as tile
from concourse import bacc, mybir
from contextlib import ExitStack

F32 = mybir.dt.float32
BF16 = mybir.dt.bfloat16
F16 = mybir.dt.float16

EPS = 1e-5
NPIX = 4 * 126 * 126  # elements per (sample, group)


def _ap(base, dims, offset):
    """Copy of AP `base` with raw [step,count] dims and element offset."""
    a = base.copy()
    a.ap = mybir.VecI64Pair([list(d) for d in dims])
    a.offset = offset
    return a


def _apf(base, free_dims, elem_offset):
    """SBUF AP: keep `base`'s partition dim, replace free dims, add offset."""
    a = base.copy()
    a.ap = mybir.VecI64Pair([list(base.ap[0])] + [list(d) for d in free_dims])
    a.offset = base.offset + elem_offset
    return a


class Pools:
    pass


def build_pools(ctx, tc):
    p = Pools()
    p.consts = ctx.enter_context(tc.tile_pool(name="consts", bufs=1))
    p.i4 = ctx.enter_context(tc.tile_pool(name="i4", bufs=2))
    p.ps = ctx.enter_context(tc.tile_pool(name="psc", bufs=3, space="PSUM"))
    p.gps = ctx.enter_context(tc.tile_pool(name="gps", bufs=1, space="PSUM"))
    p.y = ctx.enter_context(tc.tile_pool(name="ybuf", bufs=2))
    p.pool = ctx.enter_context(tc.tile_pool(name="pools", bufs=2))
    p.st = ctx.enter_context(tc.tile_pool(name="stats", bufs=2))
    return p


def load_consts(nc, p, wq_in, sel_in, bias_in, ws_in, gb_in):
    c = Pools()
    wkt = p.consts.tile([128, 384], BF16, tag="wq")
    nc.sync.dma_start(wkt[:], wq_in[:])
    c.wq = wkt
    c.sel = p.consts.tile([128, 64], F32, tag="sel")
    nc.sync.dma_start(c.sel[:], sel_in[:])
    c.cbias = p.consts.tile([128, 1], F32, tag="cbias")
    nc.sync.dma_start(c.cbias[:], bias_in[:])
    c.ws = p.consts.tile([64, 1], F32, tag="ws")
    nc.sync.dma_start(c.ws[:], ws_in[:])
    c.gb = p.consts.tile([64, 1], F32, tag="gb")
    nc.sync.dma_start(c.gb[:], gb_in[:])
    return c


def sample_body(nc, tc, p, c, x_in, y_out, n):
    AL = mybir.AluOpType
    AF = mybir.ActivationFunctionType

    # --- i4: quartered row-shifted x[n]: partition 32g+4ic+r holds
    # x[n, ic, 32g+r : 32g+r+31, :] (29 rows for g=3). Two DMAs.
    i4 = p.i4.tile([128, 3968], BF16, tag="i4")
    base = i4[:]
    pstep = base.ap[0][0]
    for g in range(4):
        nrow = 3712 if g == 3 else 3968
        dst = _ap(base, [[pstep, 32], [1, nrow]], base.offset + 32 * g * pstep)
        src = _ap(x_in, [[16384, 8], [128, 4], [1, nrow]],
                  n * 131072 + 4096 * g)
        nc.sync.dma_start(dst, src)

    # --- ybf [128, 8064] fp16 = [pair 63][par 2][w2 64]; pads (w2=63) -> 0
    ybf = p.y.tile([128, 8064], F16, tag="ybf")
    nc.vector.memset(_apf(ybf[:], [[128, 63], [64, 2]], 63), 0.0)
    sacc = p.st.tile([128, 18], F32, tag="sacc")

    # --- conv matmuls: 8 tiles = (step 2) x (quarter g 4); 4-way row-group
    # concurrency via partition bases 0/32/64/96.
    tiles = []
    for step in range(2):
        for g in range(4):
            ps = p.ps.tile([128, 1024], F32, tag="ps")
            tiles.append((ps, step, g))
            for kw in range(3):
                wk = c.wq[32 * g: 32 * g + 32, 128 * kw: 128 * kw + 128]
                for h in range(2):
                    tq = 2 * step + h
                    t_global = 4 * g + tq
                    npair = 3 if t_global == 15 else 4
                    n_t = 126 * npair
                    rhs = _apf(i4[32 * g: 32 * g + 32],
                               [[256, npair], [1, 126]], tq * 1024 + kw)
                    nc.tensor.matmul(ps[:, 512 * h: 512 * h + n_t],
                                     wk, rhs,
                                     start=(kw == 0), stop=(kw == 2),
                                     tile_position=(32 * g, 0))

    # --- ACT: 2 parity ops per tile; contiguous dst runs of 63
    acc_col = 0
    for ps, step, g in tiles:
        pair0 = 16 * g + 8 * step
        last = (g == 3 and step == 1)
        for par in range(2):
            if not last:
                in_ap = _apf(ps[:], [[512, 2], [126, 4], [2, 63]], par)
                out_ap = _apf(ybf[:], [[512, 2], [128, 4], [1, 63]],
                              128 * pair0 + 64 * par)
                nc.scalar.activation(out_ap, in_ap, AF.Identity,
                                     bias=c.cbias[:, 0:1], scale=1.0,
                                     accum_out=sacc[:, acc_col:acc_col + 1])
                acc_col += 1
            else:
                in_ap = _apf(ps[:], [[126, 4], [2, 63]], par)
                out_ap = _apf(ybf[:], [[128, 4], [1, 63]],
                              128 * pair0 + 64 * par)
                nc.scalar.activation(out_ap, in_ap, AF.Identity,
                                     bias=c.cbias[:, 0:1], scale=1.0,
                                     accum_out=sacc[:, acc_col:acc_col + 1])
                acc_col += 1
                in_ap = _apf(ps[:], [[126, 3], [2, 63]], 512 + par)
                out_ap = _apf(ybf[:], [[128, 3], [1, 63]],
                              128 * (pair0 + 4) + 64 * par)
                nc.scalar.activation(out_ap, in_ap, AF.Identity,
                                     bias=c.cbias[:, 0:1], scale=1.0,
                                     accum_out=sacc[:, acc_col:acc_col + 1])
                acc_col += 1

    stat2 = p.st.tile([128, 2], F32, tag="stat2")

    # --- maxpool: lvl1 w-pairs (2x), hpool row-pairs (2x), lvl2 w-quads,
    # partition fold
    p1 = p.pool.tile([128, 4032], F16, tag="p1")
    in0 = _apf(ybf[:], [[128, 63], [1, 64]], 0)
    in1 = _apf(ybf[:], [[128, 63], [1, 64]], 64)
    nc.vector.tensor_tensor(p1[:], in0, in1, op=AL.max)

    p2 = p.pool.tile([128, 1984], F16, tag="p2")
    in0 = _apf(p1[:], [[128, 31], [1, 64]], 0)
    in1 = _apf(p1[:], [[128, 31], [1, 64]], 64)
    nc.vector.tensor_tensor(p2[:], in0, in1, op=AL.max)

    p3 = p.pool.tile([128, 992], F16, tag="p3")
    in0 = _apf(p2[:], [[64, 31], [2, 32]], 0)
    in1 = _apf(p2[:], [[64, 31], [2, 32]], 1)
    nc.vector.tensor_tensor(p3[:], in0, in1, op=AL.max)

    # partition fold: remap upper half down via DMA, then max
    p3b = p.pool.tile([64, 992], F16, tag="p3b")
    nc.sync.dma_start(p3b[:], p3[64:128, :])
    p4 = p.pool.tile([64, 992], F16, tag="p4")
    nc.vector.tensor_tensor(p4[:], p3[0:64, :], p3b[:], op=AL.max)

    # --- sumsq in-place y*y into ybf; ordered after the pool reads (WAR)
    yv = _apf(ybf[:], [[1, 8064]], 0)
    nc.vector.scalar_tensor_tensor(yv, yv, 1.0, yv, op0=AL.mult,
                                   op1=AL.mult, accum_out=stat2[:, 1:2])

    # --- stats -> A, B
    nc.vector.reduce_sum(stat2[:, 0:1], sacc[:], axis=mybir.AxisListType.X)
    gsum = p.gps.tile([64, 2], F32, tag="gsum")
    nc.tensor.matmul(gsum[:], c.sel[:], stat2[:], start=True, stop=True)

    mv = p.st.tile([64, 2], F32, tag="mv")
    nc.vector.tensor_scalar(mv[:], gsum[:], 1.0 / NPIX, None, op0=AL.mult)
    msq = p.st.tile([64, 1], F32, tag="msq")
    nc.vector.tensor_tensor(msq[:], mv[:, 0:1], mv[:, 0:1], op=AL.mult)
    veps = p.st.tile([64, 1], F32, tag="veps")
    nc.vector.scalar_tensor_tensor(veps[:], mv[:, 1:2], EPS, msq[:],
                                   op0=AL.add, op1=AL.subtract)
    rv = p.st.tile([64, 1], F32, tag="rv")
    nc.vector.reciprocal(rv[:], veps[:])
    istd = p.st.tile([64, 1], F32, tag="istd")
    nc.scalar.activation(istd[:], rv[:], AF.Sqrt)
    aap = p.st.tile([64, 1], F32, tag="aap")
    nc.vector.tensor_tensor(aap[:], c.ws[:], istd[:], op=AL.mult)
    mua = p.st.tile([64, 1], F32, tag="mua")
    nc.vector.tensor_tensor(mua[:], mv[:, 0:1], aap[:], op=AL.mult)
    bap = p.st.tile([64, 1], F32, tag="bap")
    nc.vector.tensor_tensor(bap[:], c.gb[:], mua[:], op=AL.subtract)

    # --- finalize: A*p + B then clamp [0,1] -> fp32 via DMA cast
    fin = p.pool.tile([64, 992], F16, tag="fin")
    nc.vector.tensor_scalar(fin[:], p4[:], aap[:, 0:1], bap[:, 0:1],
                            op0=AL.mult, op1=AL.add)
    outb = p.pool.tile([64, 992], F16, tag="outb")
    nc.vector.tensor_scalar(outb[:], fin[:], 0.0, 1.0, op0=AL.max, op1=AL.min)

    dst = _ap(y_out, [[961, 64], [1, 961]], n * 61504)
    src = _apf(outb[:], [[32, 31], [1, 31]], 0)
    nc.gpsimd.dma_start(dst, src)


def build_kernel_nc(S, n_cores=8, repeat=1, use_for_i=False):
    nc = bacc.Bacc("TRN2", target_bir_lowering=False, debug=False,
                   num_devices=n_cores)
    x_in = nc.dram_tensor("x", [S, 8, 128, 128], BF16, kind="ExternalInput").ap()
    wq_in = nc.dram_tensor("wq", [128, 384], BF16, kind="ExternalInput").ap()
    sel_in = nc.dram_tensor("sel", [128, 64], F32, kind="ExternalInput").ap()
    bias_in = nc.dram_tensor("cbias", [128, 1], F32, kind="ExternalInput").ap()
    ws_in = nc.dram_tensor("ws", [64, 1], F32, kind="ExternalInput").ap()
    gb_in = nc.dram_tensor("gb", [64, 1], F32, kind="ExternalInput").ap()
    y_out = nc.dram_tensor("y", [S, 64, 31, 31], F32, kind="ExternalOutput").ap()
    with tile.TileContext(nc) as tc:
        with ExitStack() as ctx:
            p = build_pools(ctx, tc)
            c = load_consts(nc, p, wq_in, sel_in, bias_in, ws_in, gb_in)
            if use_for_i and repeat > 1:
                with tc.For_i(0, repeat, 1):
                    for n in range(S):
                        sample_body(nc, tc, p, c, x_in, y_out, n)
            else:
                for _ in range(repeat):
                    for n in range(S):
                        sample_body(nc, tc, p, c, x_in, y_out, n)
    nc.compile()
    return nc


def make_consts(conv_w, conv_b, gn_w, gn_b, scale):
    """Host-side constant assembly."""
    import ml_dtypes
    conv_w = np.asarray(conv_w, dtype=np.float32)
    # w96[ic*4+kh+j, kw*128 + oc+64j] = conv_w[oc, ic, kh, kw]; quartered 4x
    w96 = np.zeros((32, 384), np.float32)
    oc = np.arange(64)
    for kw in range(3):
        for j in range(2):
            for ic in range(8):
                for kh in range(3):
                    w96[ic * 4 + kh + j, kw * 128 + oc + 64 * j] = \
                        conv_w[oc, ic, kh, kw]
    wq = np.tile(w96, (4, 1)).astype(ml_dtypes.bfloat16)
    sel = np.zeros((128, 64), np.float32)
    for j in range(2):
        for o in range(64):
            sel[o + 64 * j, (o // 4) * 4: (o // 4) * 4 + 4] = 1.0
    cbias = np.tile(np.asarray(conv_b, np.float32).reshape(64, 1), (2, 1))
    ws = (np.asarray(gn_w, np.float32).reshape(64) *
          np.asarray(scale, np.float32).reshape(64)).reshape(64, 1)
    gb = (np.asarray(gn_b, np.float32).reshape(64) *
          np.asarray(scale, np.float32).reshape(64)).reshape(64, 1)
    return dict(wq=wq, sel=sel, cbias=cbias.astype(np.float32),
                ws=ws.astype(np.float32), gb=gb.astype(np.float32))


# ---------------------------------------------------------------------------
# Harness entry point: full (unsharded) inputs -> full output.
# ---------------------------------------------------------------------------
N_CORES = 8
S_PER_CORE = 16
_NC_CACHE = {}


def _get_nc(repeat=1, use_for_i=False):
    key = (repeat, use_for_i)
    if key not in _NC_CACHE:
        _NC_CACHE[key] = build_kernel_nc(S_PER_CORE, n_cores=N_CORES,
                                         repeat=repeat, use_for_i=use_for_i)
    return _NC_CACHE[key]


def cast_x(x):
    import ml_dtypes
    return np.ascontiguousarray(np.asarray(x, dtype=np.float32)).astype(
        ml_dtypes.bfloat16)


def kernel(x, conv_w, conv_b, gn_w, gn_b, scale):
    from concourse.bass_utils import run_bass_kernel_spmd
    xb = cast_x(x)
    consts = make_consts(conv_w, conv_b, gn_w, gn_b, scale)
    nc = _get_nc()
    in_maps = []
    for c in range(N_CORES):
        m = dict(consts)
        m["x"] = xb[c * S_PER_CORE:(c + 1) * S_PER_CORE]
        in_maps.append(m)
    res = run_bass_kernel_spmd(nc, in_maps, core_ids=list(range(N_CORES)))
    return np.concatenate([res.results[c]["y"] for c in range(N_CORES)],
                          axis=0)
